# revision 19
# baseline (speedup 1.0000x reference)
"""Trainium2 Bass kernel for nn_ExpertAttention (MoE-routed BERT-style attention).

Contract: kernel(**inputs) takes the FULL inputs from reference.setup_inputs()
and returns the FULL [64, 512, 768] fp32 output. Internally the batch is
data-parallel sharded over 8 NeuronCores; the per-expert weight stacks and
distribution stats are replicated. Routing, expert-weight selection (runtime-
predicated / dynamically-offset DMA), QKV projections, attention, and the
output projection all run on-device.

Layout notes (device):
  - X is shipped host-pre-transposed: xt[b, d, s] so the contraction dim d
    lands on SBUF partitions without an on-chip transpose.
  - Weights are shipped swizzled to [e, p, (t, j)] with d = t*128+p so a
    single DMA produces the lhsT layout for the projections.
  - Matmuls run as float32r (full fp32 operands, 1 cycle/row at N>=256),
    except the attention probabilities which are stored bf16.
  - Softmax: scores^T is built [k, q]; exp on ScalarE with the additive mask
    as a per-partition bias; Z via ones-matmul (col-tiled 4 heads/bank);
    normalization is applied to ctx^T with a broadcast reciprocal.
"""

import os
import sys

for _p in ("/opt/trn_rl_repo",):
    if os.path.isdir(_p) and _p not in sys.path:
        sys.path.insert(0, _p)

import numpy as np
import ml_dtypes

import concourse.bass as bass
import concourse.mybir as mybir
from concourse import tile
from concourse.vector_clock import ScopedClock
from concourse.bass_utils import run_bass_kernel_spmd

# Problem shapes (hardcoded per contract).
B, S, D = 64, 512, 768
H, HD = 12, 64
E, EF = 4, 3
NCORES = 8
BC = B // NCORES          # samples per core
P = 128                   # SBUF partitions
NT = S // P               # 4 s-tiles
ND = D // P               # 6 d-tiles
CHK = 384                 # N-chunk for D-wide matmul outputs (2 per 768)
THRESH = 2.0 * D * S      # z-threshold rescaled: sum_d |(sum_s x - 512*m)*rstd| < 2*768*512
OFF_WQ = 0                # packed per-expert "wall" column offsets
OFF_WK = ND * D
OFF_WV = 2 * ND * D
OFF_WO = 3 * ND * D
OFF_BQ = 4 * ND * D
OFF_BK = OFF_BQ + ND
OFF_BV = OFF_BK + ND
OFF_BO = OFF_BV + D
WALLC = OFF_BO + D

F32 = mybir.dt.float32
F32R = mybir.dt.float32r
BF16 = mybir.dt.bfloat16
FP16 = mybir.dt.float16
I32 = mybir.dt.int32
AX = mybir.AxisListType.X
OP = mybir.AluOpType
AF = mybir.ActivationFunctionType


class _TC(tile.TileContext):
    """TileContext whose emitted instructions carry at most ONE sem wait.

    The walrus build reachable from this client rejects instructions with
    more than one sync-wait command ("Too many sync wait commands").  Extra
    waits are hoisted onto same-engine nops immediately preceding the
    instruction (engines execute in order, so the semantics are identical).
    """

    def _split_waits(self, inst):
        si = getattr(inst, "sync_info", None)
        if (
            si is not None
            and si.on_wait
            and len(si.on_wait) > 1
            and inst.engine != mybir.EngineType.Unassigned
        ):
            waits = list(si.on_wait)
            inst.sync_info = mybir.SyncInfo(
                on_wait=[waits[-1]], on_update=list(si.on_update or [])
            )
            for w in waits[:-1]:
                n = self.nc.engines[inst.engine].nop(nofuse=True)
                n.ins.sync_info = mybir.SyncInfo(on_wait=[w], on_update=[])

    def _add_instruction(self, inst):
        self._split_waits(inst)
        super()._add_instruction(inst)

    def _drain_and_barrier(self, tick_clock, wait_clock):
        drain_inst = self.nc.sync.drain()
        wait_clock.add_sem_waits(
            drain_inst.ins, ScopedClock({None: tick_clock.global_clock})
        )
        si = drain_inst.ins.sync_info
        if si is not None and si.on_wait and len(si.on_wait) > 1:
            waits = list(si.on_wait)
            drain_inst.ins.sync_info = mybir.SyncInfo(
                on_wait=[waits[0]], on_update=list(si.on_update or [])
            )
            for w in waits[1:]:
                n = self.nc.sync.nop(nofuse=True)
                n.ins.sync_info = mybir.SyncInfo(on_wait=[w], on_update=[])
        self.nc.all_engine_barrier()
        assert self.sems is not None
        popped = self.nc._tile_sem_poison_stack.pop()
        assert popped is self._sem_poison
        self.nc.clear_and_free_semaphores(list(self.sems.allocated().values()))
        self.nc.all_engine_barrier()


def _r(ap):
    return ap.bitcast(F32R)


def _build_program(n_samples=BC):
    nc = bass.Bass("TRN2", target_bir_lowering=False, debug=False, num_devices=NCORES)

    xt_d = nc.dram_tensor("xt", [BC, D, S], F32R, kind="ExternalInput")
    mask_d = nc.dram_tensor("maskc", [P, BC * NT], F32, kind="ExternalInput")
    wall_d = nc.dram_tensor("wall", [E, P, WALLC], F32R, kind="ExternalInput")
    dm_d = nc.dram_tensor("dm", [P, EF * ND], F32, kind="ExternalInput")
    dr_d = nc.dram_tensor("dr", [P, EF * ND], F32, kind="ExternalInput")
    onesf_d = nc.dram_tensor("onesf", [P, 1], F32, kind="ExternalInput")
    onesh_d = nc.dram_tensor("onesh", [P, 1], BF16, kind="ExternalInput")
    ones1_d = nc.dram_tensor("ones1", [1, P], F32R, kind="ExternalInput")
    ones1h_d = nc.dram_tensor("ones1h", [1, P], FP16, kind="ExternalInput")
    out_d = nc.dram_tensor("out", [BC, S, D], F32, kind="ExternalOutput")

    from contextlib import ExitStack

    with _TC(nc) as tc, ExitStack() as ctx:
        cp = ctx.enter_context(tc.tile_pool(name="consts", bufs=1))
        wp = ctx.enter_context(tc.tile_pool(name="wcur", bufs=1))
        xtp = ctx.enter_context(tc.tile_pool(name="xt", bufs=2))
        qkp = ctx.enter_context(tc.tile_pool(name="qk", bufs=1))
        vp = ctx.enter_context(tc.tile_pool(name="v", bufs=1))
        cxsp = ctx.enter_context(tc.tile_pool(name="ctxT", bufs=1))
        ep = ctx.enter_context(tc.tile_pool(name="exp", bufs=6))
        op_ = ctx.enter_context(tc.tile_pool(name="outt", bufs=3))
        hp = ctx.enter_context(tc.tile_pool(name="small", bufs=2))
        rp = ctx.enter_context(tc.tile_pool(name="recip", bufs=8))
        zsbp = ctx.enter_context(tc.tile_pool(name="zbcsb", bufs=2))

        pp = ctx.enter_context(tc.tile_pool(name="pp", bufs=2, space="PSUM"))
        scp = ctx.enter_context(tc.tile_pool(name="scps", bufs=2, space="PSUM"))
        cxp = ctx.enter_context(tc.tile_pool(name="ctxps", bufs=2, space="PSUM"))
        zqp = ctx.enter_context(tc.tile_pool(name="zqps", bufs=1, space="PSUM"))
        zbp = ctx.enter_context(tc.tile_pool(name="zbcps", bufs=1, space="PSUM"))

        # ---- persistent constants -------------------------------------
        onesf = cp.tile([P, 1], F32)
        nc.sync.dma_start(onesf[:], onesf_d.ap())
        onesh = cp.tile([P, 1], BF16)
        nc.sync.dma_start(onesh[:], onesh_d.ap())
        ones1 = cp.tile([1, P], F32R)
        nc.sync.dma_start(ones1[:], ones1_d.ap())
        ones1h = cp.tile([1, P], FP16)
        nc.sync.dma_start(ones1h[:], ones1h_d.ap())
        dm = cp.tile([P, EF * ND], F32)
        nc.sync.dma_start(dm[:], dm_d.ap())
        dr = cp.tile([P, EF * ND], F32)
        nc.sync.dma_start(dr[:], dr_d.ap())
        maskc = cp.tile([P, BC * NT], F32)
        nc.sync.dma_start(maskc[:], mask_d.ap())
        prevf = cp.tile([1, 1], F32)
        nc.vector.memset(prevf[:], -1.0)

        # current-expert weights (persist across samples; reloaded only when
        # the routed expert changes)
        wall_c = wp.tile([P, WALLC], F32R, tag="wall")

        for s in range(n_samples):
            # ---- load X^T ---------------------------------------------
            xt = xtp.tile([P, ND * S], F32R, tag="xt")
            nc.sync.dma_start(xt[:], xt_d[s].rearrange("(t p) q -> p t q", p=P))

            # ---- routing ----------------------------------------------
            hs = hp.tile([P, ND], F32, tag="hs")
            for t in range(ND):
                nc.vector.reduce_sum(hs[:, t : t + 1], xt[:, t * S : (t + 1) * S].bitcast(F32), axis=AX)
            zc = hp.tile([P, EF], F32, tag="zc")
            ztmp = hp.tile([P, ND], F32, tag="ztmp")
            for e in range(EF):
                nc.vector.tensor_tensor(ztmp[:], hs[:], dm[:, e * ND : (e + 1) * ND], op=OP.subtract)
                nc.vector.tensor_tensor(ztmp[:], ztmp[:], dr[:, e * ND : (e + 1) * ND], op=OP.mult)
                nc.vector.tensor_reduce(
                    zc[:, e : e + 1], ztmp[:], axis=AX, op=OP.add, apply_absolute_value=True
                )
            zs_ps = zqp.tile([1, EF], F32, tag="zq")
            nc.tensor.matmul(zs_ps[:], onesf[:, 0:1], zc[:], start=True, stop=True)
            nf = hp.tile([1, EF], F32, tag="nf")
            nc.vector.tensor_scalar(nf[:], zs_ps[:], THRESH, None, op0=OP.is_ge)
            t1 = hp.tile([1, 1], F32, tag="t1")
            t2 = hp.tile([1, 1], F32, tag="t2")
            asgf = hp.tile([1, 1], F32, tag="asgf")
            needf = hp.tile([1, 1], F32, tag="needf")
            nc.vector.tensor_tensor(t1[:], nf[:, 0:1], nf[:, 1:2], op=OP.mult)
            nc.vector.tensor_tensor(t2[:], t1[:], nf[:, 2:3], op=OP.mult)
            nc.vector.tensor_tensor(asgf[:], nf[:, 0:1], t1[:], op=OP.add)
            nc.vector.tensor_tensor(asgf[:], asgf[:], t2[:], op=OP.add)
            nc.vector.tensor_tensor(needf[:], asgf[:], prevf[:], op=OP.not_equal)
            nc.vector.tensor_copy(prevf[:], asgf[:])
            # effective fetch index: routed expert, or far out-of-bounds when
            # the expert is unchanged (the DMA is then skipped entirely)
            efff = hp.tile([1, 1], F32, tag="efff")
            effi = hp.tile([1, 1], I32, tag="effi")
            nc.vector.tensor_tensor(efff[:], asgf[:], needf[:], op=OP.mult)
            nc.vector.tensor_scalar(
                needf[:], needf[:], -1000.0, 1000.0, op0=OP.mult, op1=OP.add
            )
            nc.vector.tensor_tensor(efff[:], efff[:], needf[:], op=OP.add)
            nc.vector.tensor_copy(effi[:], efff[:])
            eff = nc.values_load(
                effi[0:1, 0:1],
                engines=[mybir.EngineType.SP],
                # claimed bounds cover only the valid range; the "unchanged"
                # sentinel (1000+) trips the runtime OOB check, which skips
                # the whole DMA (bounds_check="skip_entire_dma")
                min_val=0,
                max_val=3,
                skip_runtime_bounds_check=True,
            )

            # ---- expert weight+bias fetch (skipped when unchanged) ----
            nc.sync.dma_start(
                wall_c[:], wall_d[bass.ds(eff, 1)], bounds_check="skip_entire_dma"
            )

            # ---- projections q^T, k^T ---------------------------------
            qT = qkp.tile([P, ND * S], F32R, tag="qT")
            kT = qkp.tile([P, ND * S], F32R, tag="kT")
            for w0, b0, dst in ((0, OFF_BQ, qT), (ND * D, OFF_BK, kT)):
                for tp in range(ND):
                    ps = pp.tile([P, S], F32, tag="pp")
                    for k in range(ND):
                        nc.tensor.matmul(
                            ps[:],
                            wall_c[:, w0 + k * D + tp * P : w0 + k * D + tp * P + P],
                            xt[:, k * S : (k + 1) * S],
                            start=(k == 0),
                            stop=(k == ND - 1),
                        )
                    # add per-partition bias during PSUM->SBUF move (DVE)
                    nc.vector.tensor_scalar(
                        dst[:, tp * S : (tp + 1) * S],
                        ps[:],
                        wall_c[:, b0 + tp : b0 + tp + 1].bitcast(F32),
                        None,
                        op0=OP.add,
                    )

            # ---- projection v (natural layout, bf16) ------------------
            v_sb = vp.tile([P, NT * D], BF16, tag="v")
            for m in range(NT):
                for ec in range(2):
                    ps = pp.tile([P, CHK], F32, tag="pp")
                    for k in range(ND):
                        nc.tensor.matmul(
                            ps[:],
                            xt[:, k * S + m * P : k * S + m * P + P],
                            wall_c[:, OFF_WV + k * D + ec * CHK : OFF_WV + k * D + (ec + 1) * CHK],
                            start=(k == 0),
                            stop=(k == ND - 1),
                        )
                    # add bv (replicated rows) and cast to bf16
                    nc.vector.tensor_tensor(
                        v_sb[:, m * D + ec * CHK : m * D + (ec + 1) * CHK],
                        ps[:],
                        wall_c[:, OFF_BV + ec * CHK : OFF_BV + (ec + 1) * CHK].bitcast(F32),
                        op=OP.add,
                    )

            # ---- attention (3 quads of 4 heads) -----------------------
            ctxT = cxsp.tile([P, ND * S], F32R, tag="ctxT")
            for j in range(3):
                zq_ps = zqp.tile([P, S], F32, tag="zq")
                exp_t = []
                for hh in range(4):
                    h = 4 * j + hh
                    tp = h // 2
                    half = (h % 2) * 64
                    e_sb = ep.tile([P, NT * S], BF16, tag="exp")
                    exp_t.append(e_sb)
                    for c in range(NT):
                        sc_ps = scp.tile([P, S], F32, tag="sc")
                        nc.tensor.matmul(
                            sc_ps[:],
                            kT[half : half + 64, tp * S + c * P : tp * S + c * P + P],
                            qT[half : half + 64, tp * S : (tp + 1) * S],
                            start=True,
                            stop=True,
                            tile_position=(half, 0),
                        )
                        # exp((q.k)/8 + mask_k) with mask as per-partition bias
                        nc.scalar.activation(
                            e_sb[:, c * S : (c + 1) * S],
                            sc_ps[:],
                            AF.Exp,
                            bias=maskc[:, s * NT + c : s * NT + c + 1],
                            scale=0.125,
                        )
                        nc.tensor.matmul(
                            zq_ps[32 * hh : 32 * hh + 1, :],
                            onesh[:, 0:1],
                            e_sb[:, c * S : (c + 1) * S],
                            start=(c == 0),
                            stop=(c == NT - 1),
                            tile_position=(0, 32 * hh),
                            skip_group_check=True,
                        )
                recips = []
                for hh in range(4):
                    rc = rp.tile([1, S], FP16, tag="recip")
                    with nc.allow_low_precision(reason="f32r is fp32-width"):
                        nc.vector.reciprocal(rc[:], zq_ps[32 * hh : 32 * hh + 1, :])
                    recips.append(rc)
                for ii in range(2):
                    zbc_ps = zbp.tile([P, S], F32, tag="zbc")
                    for half, hh in ((0, 2 * ii), (64, 2 * ii + 1)):
                        nc.tensor.matmul(
                            zbc_ps[half : half + 64, :],
                            ones1h[0:1, 0:64],
                            recips[hh][0:1, :],
                            start=True,
                            stop=True,
                            tile_position=(0, half),
                            skip_group_check=True,
                        )
                    zbc_sb = zsbp.tile([P, S], F32, tag="zbcsb")
                    nc.scalar.copy(zbc_sb[:], zbc_ps[:])
                    ctx_ps = cxp.tile([P, S], F32, tag="ctx")
                    for c in range(NT):
                        for half, hh in ((0, 2 * ii), (64, 2 * ii + 1)):
                            h = 4 * j + hh
                            nc.tensor.matmul(
                                ctx_ps[half : half + 64, :],
                                v_sb[:, c * D + h * 64 : c * D + h * 64 + 64],
                                exp_t[hh][:, c * S : (c + 1) * S],
                                start=(c == 0),
                                stop=(c == NT - 1),
                                tile_position=(0, half),
                                skip_group_check=True,
                            )
                    nc.vector.tensor_tensor(
                        ctxT[:, (2 * j + ii) * S : (2 * j + ii + 1) * S],
                        ctx_ps[:],
                        zbc_sb[:],
                        op=OP.mult,
                    )

            # ---- output projection ------------------------------------
            for m in range(NT):
                ot = op_.tile([P, D], F32, tag="ot")
                for ec in range(2):
                    ps = pp.tile([P, CHK], F32, tag="pp")
                    for tp in range(ND):
                        nc.tensor.matmul(
                            ps[:],
                            ctxT[:, tp * S + m * P : tp * S + m * P + P],
                            wall_c[:, OFF_WO + tp * D + ec * CHK : OFF_WO + tp * D + (ec + 1) * CHK],
                            start=(tp == 0),
                            stop=(tp == ND - 1),
                        )
                    # add bo (replicated rows) during PSUM->SBUF move
                    nc.vector.tensor_tensor(
                        ot[:, ec * CHK : (ec + 1) * CHK],
                        ps[:],
                        wall_c[:, OFF_BO + ec * CHK : OFF_BO + (ec + 1) * CHK].bitcast(F32),
                        op=OP.add,
                    )
                nc.sync.dma_start(out_d[s, m * P : (m + 1) * P, :], ot[:])

    return nc


_PROGRAM = None


def _get_program():
    global _PROGRAM
    if _PROGRAM is None:
        _PROGRAM = _build_program(int(os.environ.get("KBENCH_SAMPLES", BC)))
    return _PROGRAM


def _prep_inputs(inputs):
    f32 = np.float32
    ht = np.asarray(inputs["hidden_states"], f32)
    mask = np.asarray(inputs["attention_mask"], f32)
    Wq = np.asarray(inputs["Wq"], f32)
    Wk = np.asarray(inputs["Wk"], f32)
    Wv = np.asarray(inputs["Wv"], f32)
    Wo = np.asarray(inputs["Wo"], f32)
    bq = np.asarray(inputs["bq"], f32)
    bk = np.asarray(inputs["bk"], f32)
    bv = np.asarray(inputs["bv"], f32)
    bo = np.asarray(inputs["bo"], f32)
    dmean = np.asarray(inputs["dist_mean"], f32)
    dstd = np.asarray(inputs["dist_std"], f32)

    def sw_w(W):
        return W.reshape(E, ND, P, D).transpose(0, 2, 1, 3).reshape(E, P, ND * D)

    def sw_b(b):
        return b.reshape(E, ND, P).transpose(0, 2, 1)

    def sw_d(a):
        return np.ascontiguousarray(a.reshape(EF, ND, P).transpose(2, 0, 1).reshape(P, EF * ND))

    wall = np.empty((E, P, WALLC), f32)
    wall[:, :, OFF_WQ : OFF_WQ + ND * D] = sw_w(Wq)
    wall[:, :, OFF_WK : OFF_WK + ND * D] = sw_w(Wk)
    wall[:, :, OFF_WV : OFF_WV + ND * D] = sw_w(Wv)
    wall[:, :, OFF_WO : OFF_WO + ND * D] = sw_w(Wo)
    wall[:, :, OFF_BQ : OFF_BQ + ND] = sw_b(bq)
    wall[:, :, OFF_BK : OFF_BK + ND] = sw_b(bk)
    wall[:, :, OFF_BV : OFF_BV + D] = bv[:, None, :]
    wall[:, :, OFF_BO : OFF_BO + D] = bo[:, None, :]

    shared = {
        "wall": wall,
        "dm": sw_d(512.0 * dmean),
        "dr": sw_d((1.0 / dstd).astype(f32)),
        "onesf": np.ones((P, 1), f32),
        "onesh": np.ones((P, 1), ml_dtypes.bfloat16),
        "ones1": np.ones((1, P), f32),
        "ones1h": np.ones((1, P), np.float16),
    }
    xt_all = np.ascontiguousarray(ht.transpose(0, 2, 1))  # [B, D, S]
    in_maps = []
    for c in range(NCORES):
        mc = mask[c * BC : (c + 1) * BC]  # [BC, S]
        maskc = np.ascontiguousarray(
            mc.reshape(BC, NT, P).transpose(2, 0, 1).reshape(P, BC * NT)
        )
        m = dict(shared)
        m["xt"] = np.ascontiguousarray(xt_all[c * BC : (c + 1) * BC])
        m["maskc"] = maskc
        in_maps.append(m)
    return in_maps


def _run(inputs, trace=False, trace_cores=None):
    nc = _get_program()
    in_maps = _prep_inputs(inputs)
    res = run_bass_kernel_spmd(
        nc,
        in_maps,
        core_ids=list(range(NCORES)),
        trace=trace,
        trace_cores=trace_cores,
    )
    out = np.concatenate([res.results[c]["out"] for c in range(NCORES)], axis=0)
    return out, res


def kernel(**inputs) -> np.ndarray:
    out, _ = _run(inputs)
    return out
